# revision 5
# baseline (speedup 1.0000x reference)
"""Distributed matvec kernel for nn_CubicalModel_ISM.

Computes Xp = I @ p, Yp = J @ p with I, J: [784, 50000], p: [50000], then
gathers tiny [50, 2] persistence diagrams from the 28x28 reshapes.

Strategy (8 NeuronCores):
  - Shard the contraction dim P=50000 column-wise across 8 cores
    (6272 = 49*128 per core, last core zero-padded).
  - Host-side: transpose each shard to [K, 784] and split fp32 into
    bf16 hi + bf16 lo planes (same total bytes as fp32, so the memory
    roofline is unchanged, but the PE runs at bf16 rate instead of the
    4x-slower fp32 mode). p is split the same way; products
    hi*hi + hi*lo + lo*hi are accumulated in fp32 PSUM, recovering
    fp32-level precision (dropped lo*lo term is ~2^-18 relative).
  - Each core streams its 39.2 MB of matrix data HBM->SBUF (HWDGE via
    the SP sequencer, one queue, in-order) and accumulates 6 partial
    row-vectors ([784] each) in PSUM via PE matmuls with p-columns as
    the stationary operand.
  - Raw Bass (no Tile): this walrus build only supports ONE sync-wait
    per DMA instruction, so waits are emitted as standalone engine
    wait_ge ops and DMAs carry none.
  - Host: sum the 8 cores' partials (the "all-reduce"), reshape, gather.
"""

import numpy as np
import ml_dtypes

import concourse.bass as bass
import concourse.mybir as mybir
from concourse.bass_utils import run_bass_kernel_spmd

N_CORES = 8
P_FULL = 50000
H = W = 28
M = H * W  # 784
KT = 49  # k-tiles (of 128) per core
K_PER = KT * 128  # 6272
NHALF = 392  # 784 / 2, per-PSUM-bank output chunk

BF16 = ml_dtypes.bfloat16
F32 = np.float32

B = 12  # stream buffers per tag (4 tags x B x 1568B/partition SBUF)


def build_nc() -> bass.Bass:
    f32 = mybir.dt.float32
    bf16 = mybir.dt.bfloat16
    # detect_race_conditions=False: CoreSim's race detector doesn't model
    # HWDGE per-queue in-order completion, which our single-queue count
    # semaphore relies on (16 engine-incs per DMA, per-engine FIFO).
    nc = bass.Bass("TRN2", detect_race_conditions=False)
    pw_d = nc.dram_tensor("pw", [128, 2 * KT], bf16, kind="ExternalInput")
    planes_d = {
        name: nc.dram_tensor(name, [K_PER, M], bf16, kind="ExternalInput")
        for name in ("ihi", "ilo", "jhi", "jlo")
    }
    out_d = nc.dram_tensor("out", [6, M], f32, kind="ExternalOutput")

    tiled = {
        name: t[:, :].rearrange("(n p) m -> n p m", p=128)
        for name, t in planes_d.items()
    }

    from contextlib import ExitStack

    with ExitStack() as stk:
        pw_sb = stk.enter_context(nc.sbuf_tensor("pw_sb", [128, 2 * KT], bf16))
        s_ihi = stk.enter_context(nc.sbuf_tensor("s_ihi", [128, B * M], bf16))
        s_ilo = stk.enter_context(nc.sbuf_tensor("s_ilo", [128, B * M], bf16))
        s_jhi = stk.enter_context(nc.sbuf_tensor("s_jhi", [128, B * M], bf16))
        s_jlo = stk.enter_context(nc.sbuf_tensor("s_jlo", [128, B * M], bf16))
        o_ih = stk.enter_context(nc.sbuf_tensor("o_ih", [2, M], f32))
        o_il = stk.enter_context(nc.sbuf_tensor("o_il", [1, M], f32))
        o_jh = stk.enter_context(nc.sbuf_tensor("o_jh", [2, M], f32))
        o_jl = stk.enter_context(nc.sbuf_tensor("o_jl", [1, M], f32))
        ps_ih0 = stk.enter_context(nc.psum_tensor("ps_ih0", [2, NHALF], f32))
        ps_ih1 = stk.enter_context(nc.psum_tensor("ps_ih1", [2, NHALF], f32))
        ps_il0 = stk.enter_context(nc.psum_tensor("ps_il0", [1, NHALF], f32))
        ps_il1 = stk.enter_context(nc.psum_tensor("ps_il1", [1, NHALF], f32))
        ps_jh0 = stk.enter_context(nc.psum_tensor("ps_jh0", [2, NHALF], f32))
        ps_jh1 = stk.enter_context(nc.psum_tensor("ps_jh1", [2, NHALF], f32))
        ps_jl0 = stk.enter_context(nc.psum_tensor("ps_jl0", [1, NHALF], f32))
        ps_jl1 = stk.enter_context(nc.psum_tensor("ps_jl1", [1, NHALF], f32))
        dma_in = stk.enter_context(nc.semaphore("dma_in"))
        pe_sem = stk.enter_context(nc.semaphore("pe_sem"))
        dve_sem = stk.enter_context(nc.semaphore("dve_sem"))
        dma_out = stk.enter_context(nc.semaphore("dma_out"))
        block = stk.enter_context(nc.Block())
        streams = {"ihi": s_ihi, "ilo": s_ilo, "jhi": s_jhi, "jlo": s_jlo}
        ps = {
            ("i", "h"): (ps_ih0, ps_ih1),
            ("i", "l"): (ps_il0, ps_il1),
            ("j", "h"): (ps_jh0, ps_jh1),
            ("j", "l"): (ps_jl0, ps_jl1),
        }
        outs = {("i", "h"): o_ih, ("i", "l"): o_il,
                ("j", "h"): o_jh, ("j", "l"): o_jl}

        def slot_cols(n):
            s = (n % B) * M
            return slice(s, s + M)

        @block.sync
        def _(sync):
            sync.dma_start(pw_sb[:, :], pw_d[:, :]).then_inc(dma_in, 16)
            for n in range(KT):
                if n >= B:
                    # slot n%B was last used by k-tile n-B; wait until the
                    # PE has consumed it (pe_sem counts finished k-tiles)
                    sync.wait_ge(pe_sem, n - B + 1)
                cols = slot_cols(n)
                for name in ("ihi", "ilo", "jhi", "jlo"):
                    sync.dma_start(
                        streams[name][:, cols], tiled[name][n, :, :]
                    ).then_inc(dma_in, 16)
            # evict results once the DVE has drained all PSUMs
            sync.wait_ge(dve_sem, 1)
            sync.dma_start(out_d[0:2, :], o_ih[:, :]).then_inc(dma_out, 16)
            sync.dma_start(out_d[2:3, :], o_il[:, :]).then_inc(dma_out, 16)
            sync.dma_start(out_d[3:5, :], o_jh[:, :]).then_inc(dma_out, 16)
            sync.dma_start(out_d[5:6, :], o_jl[:, :]).then_inc(dma_out, 16)
            sync.wait_ge(dma_out, 64)

        @block.tensor
        def _(tensor):
            for n in range(KT):
                # pw (16) + all 4 stream DMAs of k-tiles 0..n (64 each)
                tensor.wait_ge(dma_in, 16 + 64 * (n + 1))
                cols = slot_cols(n)
                start = n == 0
                stop = n == KT - 1
                w2 = pw_sb[:, 2 * n : 2 * n + 2]  # [128, 2] (p_hi, p_lo)
                w1 = pw_sb[:, 2 * n : 2 * n + 1]  # [128, 1] (p_hi)
                last = None
                for mat in ("i", "j"):
                    hi_t = streams[f"{mat}hi"][:, cols]
                    lo_t = streams[f"{mat}lo"][:, cols]
                    for c in range(2):
                        cs = slice(cols.start + c * NHALF,
                                   cols.start + (c + 1) * NHALF)
                        last = nc.tensor.matmul(
                            ps[(mat, "h")][c][:, :], w2,
                            streams[f"{mat}hi"][:, cs],
                            start=start, stop=stop,
                        )
                        last = nc.tensor.matmul(
                            ps[(mat, "l")][c][:, :], w1,
                            streams[f"{mat}lo"][:, cs],
                            start=start, stop=stop,
                        )
                last.then_inc(pe_sem, 1)

        @block.vector
        def _(vector):
            vector.wait_ge(pe_sem, KT)
            last = None
            for mat in ("i", "j"):
                for hl in ("h", "l"):
                    for c in range(2):
                        cs = slice(c * NHALF, (c + 1) * NHALF)
                        last = nc.vector.tensor_copy(
                            outs[(mat, hl)][:, cs], ps[(mat, hl)][c][:, :]
                        )
            last.then_inc(dve_sem, 1)

    return nc


_NC_CACHE = None


def get_nc() -> bass.Bass:
    global _NC_CACHE
    if _NC_CACHE is None:
        _NC_CACHE = build_nc()
    return _NC_CACHE


def _split_hi_lo(a32: np.ndarray):
    hi = a32.astype(BF16)
    lo = (a32 - hi.astype(F32)).astype(BF16)
    return hi, lo


def shard_inputs(p, I, J) -> list[dict]:
    p = np.asarray(p, dtype=F32)
    I = np.asarray(I, dtype=F32)
    J = np.asarray(J, dtype=F32)

    p_pad = np.zeros(N_CORES * K_PER, dtype=F32)
    p_pad[:P_FULL] = p

    in_maps = []
    for c in range(N_CORES):
        lo_k = c * K_PER
        hi_k = min(lo_k + K_PER, P_FULL)
        kc = hi_k - lo_k

        pc = p_pad[c * K_PER : (c + 1) * K_PER]
        phi, plo = _split_hi_lo(pc)
        pw = np.zeros((128, 2 * KT), dtype=BF16)
        pw[:, 0::2] = phi.reshape(KT, 128).T
        pw[:, 1::2] = plo.reshape(KT, 128).T

        im = {"pw": pw}
        for name, mat in (("i", I), ("j", J)):
            t = np.ascontiguousarray(mat[:, lo_k:hi_k].T)  # [kc, 784] f32
            hi_p, lo_p = _split_hi_lo(t)
            if kc < K_PER:
                pad_hi = np.zeros((K_PER, M), dtype=BF16)
                pad_lo = np.zeros((K_PER, M), dtype=BF16)
                pad_hi[:kc] = hi_p
                pad_lo[:kc] = lo_p
                hi_p, lo_p = pad_hi, pad_lo
            im[f"{name}hi"] = hi_p
            im[f"{name}lo"] = lo_p
        in_maps.append(im)
    return in_maps


def run(p, I, J, inds1, inds2, trace=False, **run_kwargs):
    """Returns ((dgm1, dgm2), BassKernelResults)."""
    in_maps = shard_inputs(p, I, J)
    nc = get_nc()
    res = run_bass_kernel_spmd(
        nc, in_maps, list(range(N_CORES)), trace=trace, **run_kwargs
    )
    acc = np.zeros((6, M), dtype=np.float64)
    for r in res.results:
        acc += r["out"].astype(np.float64)
    Xp = (acc[0] + acc[1] + acc[2]).astype(F32).reshape(H, W)
    Yp = (acc[3] + acc[4] + acc[5]).astype(F32).reshape(H, W)
    inds1 = np.asarray(inds1)
    inds2 = np.asarray(inds2)
    dgm1 = Xp[inds1[:, 0], inds1[:, 1]].reshape(-1, 2)
    dgm2 = Yp[inds2[:, 0], inds2[:, 1]].reshape(-1, 2)
    return (dgm1, dgm2), res


def kernel(p, I, J, inds1, inds2):
    out, _ = run(p, I, J, inds1, inds2, trace=False)
    return out


# revision 8
# speedup vs baseline: 1.0069x; 1.0069x over previous
"""Distributed matvec kernel for nn_CubicalModel_ISM.

Computes Xp = I @ p, Yp = J @ p with I, J: [784, 50000], p: [50000], then
gathers tiny [50, 2] persistence diagrams from the 28x28 reshapes.

Strategy (8 NeuronCores):
  - Shard the contraction dim P=50000 column-wise across 8 cores
    (6272 = 49*128 per core, last core zero-padded).
  - Host-side: transpose each shard to [K, 784] and split fp32 into
    bf16 hi + bf16 lo planes (same total bytes as fp32, so the memory
    roofline is unchanged, but the PE runs at bf16 rate instead of the
    4x-slower fp32 mode). p is split the same way; products
    hi*hi + hi*lo + lo*hi are accumulated in fp32 PSUM, recovering
    fp32-level precision (dropped lo*lo term is ~2^-18 relative).
  - Each core streams its 39.2 MB of matrix data HBM->SBUF (HWDGE via
    the SP sequencer, one queue, in-order) and accumulates 6 partial
    row-vectors ([784] each) in PSUM via PE matmuls with p-columns as
    the stationary operand.
  - Raw Bass (no Tile): this walrus build only supports ONE sync-wait
    per DMA instruction, so waits are emitted as standalone engine
    wait_ge ops and DMAs carry none.
  - Host: sum the 8 cores' partials (the "all-reduce"), reshape, gather.
"""

import numpy as np
import ml_dtypes

import concourse.bass as bass
import concourse.mybir as mybir
from concourse.bass_utils import run_bass_kernel_spmd

N_CORES = 8
P_FULL = 50000
H = W = 28
M = H * W  # 784
KT = 49  # k-tiles (of 128) per core
K_PER = KT * 128  # 6272
NHALF = 392  # 784 / 2, per-PSUM-bank output chunk

BF16 = ml_dtypes.bfloat16
F32 = np.float32

B = 12  # stream buffers per tag (4 tags x B x 1568B/partition SBUF)


def build_nc() -> bass.Bass:
    f32 = mybir.dt.float32
    bf16 = mybir.dt.bfloat16
    nc = bass.Bass("TRN2")
    pw_d = nc.dram_tensor("pw", [128, 2 * KT], bf16, kind="ExternalInput")
    planes_d = {
        name: nc.dram_tensor(name, [K_PER, M], bf16, kind="ExternalInput")
        for name in ("ihi", "ilo", "jhi", "jlo")
    }
    out_d = nc.dram_tensor("out", [6, M], f32, kind="ExternalOutput")

    tiled = {
        name: t[:, :].rearrange("(n p) m -> n p m", p=128)
        for name, t in planes_d.items()
    }

    from contextlib import ExitStack

    with ExitStack() as stk:
        pw_sb = stk.enter_context(nc.sbuf_tensor("pw_sb", [128, 2 * KT], bf16))
        s_ihi = stk.enter_context(nc.sbuf_tensor("s_ihi", [128, B * M], bf16))
        s_ilo = stk.enter_context(nc.sbuf_tensor("s_ilo", [128, B * M], bf16))
        s_jhi = stk.enter_context(nc.sbuf_tensor("s_jhi", [128, B * M], bf16))
        s_jlo = stk.enter_context(nc.sbuf_tensor("s_jlo", [128, B * M], bf16))
        o_ih = stk.enter_context(nc.sbuf_tensor("o_ih", [2, M], f32))
        o_il = stk.enter_context(nc.sbuf_tensor("o_il", [1, M], f32))
        o_jh = stk.enter_context(nc.sbuf_tensor("o_jh", [2, M], f32))
        o_jl = stk.enter_context(nc.sbuf_tensor("o_jl", [1, M], f32))
        ps_ih0 = stk.enter_context(nc.psum_tensor("ps_ih0", [2, NHALF], f32))
        ps_ih1 = stk.enter_context(nc.psum_tensor("ps_ih1", [2, NHALF], f32))
        ps_il0 = stk.enter_context(nc.psum_tensor("ps_il0", [1, NHALF], f32))
        ps_il1 = stk.enter_context(nc.psum_tensor("ps_il1", [1, NHALF], f32))
        ps_jh0 = stk.enter_context(nc.psum_tensor("ps_jh0", [2, NHALF], f32))
        ps_jh1 = stk.enter_context(nc.psum_tensor("ps_jh1", [2, NHALF], f32))
        ps_jl0 = stk.enter_context(nc.psum_tensor("ps_jl0", [1, NHALF], f32))
        ps_jl1 = stk.enter_context(nc.psum_tensor("ps_jl1", [1, NHALF], f32))
        lanes = [
            stk.enter_context(nc.semaphore(f"dml{q}")) for q in range(8)
        ]
        pe_sem = stk.enter_context(nc.semaphore("pe_sem"))
        dve_sem = stk.enter_context(nc.semaphore("dve_sem"))
        block = stk.enter_context(nc.Block())
        streams = {"ihi": s_ihi, "ilo": s_ilo, "jhi": s_jhi, "jlo": s_jlo}
        ps = {
            ("i", "h"): (ps_ih0, ps_ih1),
            ("i", "l"): (ps_il0, ps_il1),
            ("j", "h"): (ps_jh0, ps_jh1),
            ("j", "l"): (ps_jl0, ps_jl1),
        }
        outs = {("i", "h"): o_ih, ("i", "l"): o_il,
                ("j", "h"): o_jh, ("j", "l"): o_jl}

        def slot_cols(n):
            s = (n % B) * M
            return slice(s, s + M)

        # Round-robin lane bookkeeping. Every DMA goes to lane (issue mod 8),
        # increments that lane's sem by 16, and carries exactly ONE embedded
        # wait: its own lane's previous count. That strictly orders each
        # lane's sem updates (race-free counts) while keeping 8 DMAs in
        # flight, and stays within walrus's one-sync-wait-per-DMA limit.
        lane_state = {"k": 0, "counts": [0] * 8}
        # per (kind, n): list of (lane_idx, value_after) for consumer waits
        dma_records = {}

        def issue_dma(sync, dst, src, record_key):
            q = lane_state["k"] % 8
            lane_state["k"] += 1
            prev = lane_state["counts"][q]
            ins = sync.dma_start(dst, src).then_inc(lanes[q], 16)
            if prev > 0:
                ins.wait_op(lanes[q], 16 * prev, "sem-ge")
            lane_state["counts"][q] = prev + 1
            dma_records.setdefault(record_key, []).append((q, 16 * (prev + 1)))

        @block.sync
        def _(sync):
            issue_dma(sync, pw_sb[:, :], pw_d[:, :], ("pw",))
            for n in range(KT):
                if n >= B:
                    # slot n%B was last used by k-tile n-B; wait until the
                    # PE has consumed it (pe_sem counts finished k-tiles)
                    sync.wait_ge(pe_sem, n - B + 1)
                cols = slot_cols(n)
                for name in ("ihi", "ilo", "jhi", "jlo"):
                    issue_dma(
                        sync, streams[name][:, cols], tiled[name][n, :, :],
                        ("tile", n),
                    )
            # evict results once the DVE has drained all PSUMs
            sync.wait_ge(dve_sem, 1)
            issue_dma(sync, out_d[0:2, :], o_ih[:, :], ("out",))
            issue_dma(sync, out_d[2:3, :], o_il[:, :], ("out",))
            issue_dma(sync, out_d[3:5, :], o_jh[:, :], ("out",))
            issue_dma(sync, out_d[5:6, :], o_jl[:, :], ("out",))
            for q, v in dma_records[("out",)]:
                sync.wait_ge(lanes[q], v)

        @block.tensor
        def _(tensor):
            for n in range(KT):
                if n == 0:
                    for q, v in dma_records[("pw",)]:
                        tensor.wait_ge(lanes[q], v)
                for q, v in dma_records[("tile", n)]:
                    tensor.wait_ge(lanes[q], v)
                cols = slot_cols(n)
                start = n == 0
                stop = n == KT - 1
                w2 = pw_sb[:, 2 * n : 2 * n + 2]  # [128, 2] (p_hi, p_lo)
                w1 = pw_sb[:, 2 * n : 2 * n + 1]  # [128, 1] (p_hi)
                last = None
                for mat in ("i", "j"):
                    hi_t = streams[f"{mat}hi"][:, cols]
                    lo_t = streams[f"{mat}lo"][:, cols]
                    for c in range(2):
                        cs = slice(cols.start + c * NHALF,
                                   cols.start + (c + 1) * NHALF)
                        last = nc.tensor.matmul(
                            ps[(mat, "h")][c][:, :], w2,
                            streams[f"{mat}hi"][:, cs],
                            start=start, stop=stop,
                        )
                        last = nc.tensor.matmul(
                            ps[(mat, "l")][c][:, :], w1,
                            streams[f"{mat}lo"][:, cs],
                            start=start, stop=stop,
                        )
                last.then_inc(pe_sem, 1)

        @block.vector
        def _(vector):
            vector.wait_ge(pe_sem, KT)
            last = None
            for mat in ("i", "j"):
                for hl in ("h", "l"):
                    for c in range(2):
                        cs = slice(c * NHALF, (c + 1) * NHALF)
                        last = nc.vector.tensor_copy(
                            outs[(mat, hl)][:, cs], ps[(mat, hl)][c][:, :]
                        )
            last.then_inc(dve_sem, 1)

    return nc


_NC_CACHE = None


def get_nc() -> bass.Bass:
    global _NC_CACHE
    if _NC_CACHE is None:
        _NC_CACHE = build_nc()
    return _NC_CACHE


def _split_hi_lo(a32: np.ndarray):
    hi = a32.astype(BF16)
    lo = (a32 - hi.astype(F32)).astype(BF16)
    return hi, lo


def shard_inputs(p, I, J) -> list[dict]:
    p = np.asarray(p, dtype=F32)
    I = np.asarray(I, dtype=F32)
    J = np.asarray(J, dtype=F32)

    p_pad = np.zeros(N_CORES * K_PER, dtype=F32)
    p_pad[:P_FULL] = p

    in_maps = []
    for c in range(N_CORES):
        lo_k = c * K_PER
        hi_k = min(lo_k + K_PER, P_FULL)
        kc = hi_k - lo_k

        pc = p_pad[c * K_PER : (c + 1) * K_PER]
        phi, plo = _split_hi_lo(pc)
        pw = np.zeros((128, 2 * KT), dtype=BF16)
        pw[:, 0::2] = phi.reshape(KT, 128).T
        pw[:, 1::2] = plo.reshape(KT, 128).T

        im = {"pw": pw}
        for name, mat in (("i", I), ("j", J)):
            t = np.ascontiguousarray(mat[:, lo_k:hi_k].T)  # [kc, 784] f32
            hi_p, lo_p = _split_hi_lo(t)
            if kc < K_PER:
                pad_hi = np.zeros((K_PER, M), dtype=BF16)
                pad_lo = np.zeros((K_PER, M), dtype=BF16)
                pad_hi[:kc] = hi_p
                pad_lo[:kc] = lo_p
                hi_p, lo_p = pad_hi, pad_lo
            im[f"{name}hi"] = hi_p
            im[f"{name}lo"] = lo_p
        in_maps.append(im)
    return in_maps


def run(p, I, J, inds1, inds2, trace=False, **run_kwargs):
    """Returns ((dgm1, dgm2), BassKernelResults)."""
    in_maps = shard_inputs(p, I, J)
    nc = get_nc()
    res = run_bass_kernel_spmd(
        nc, in_maps, list(range(N_CORES)), trace=trace, **run_kwargs
    )
    acc = np.zeros((6, M), dtype=np.float64)
    for r in res.results:
        acc += r["out"].astype(np.float64)
    Xp = (acc[0] + acc[1] + acc[2]).astype(F32).reshape(H, W)
    Yp = (acc[3] + acc[4] + acc[5]).astype(F32).reshape(H, W)
    inds1 = np.asarray(inds1)
    inds2 = np.asarray(inds2)
    dgm1 = Xp[inds1[:, 0], inds1[:, 1]].reshape(-1, 2)
    dgm2 = Yp[inds2[:, 0], inds2[:, 1]].reshape(-1, 2)
    return (dgm1, dgm2), res


def kernel(p, I, J, inds1, inds2):
    out, _ = run(p, I, J, inds1, inds2, trace=False)
    return out


# revision 9
# speedup vs baseline: 1.2178x; 1.2095x over previous
"""Distributed matvec kernel for nn_CubicalModel_ISM.

Computes Xp = I @ p, Yp = J @ p with I, J: [784, 50000], p: [50000], then
gathers tiny [50, 2] persistence diagrams from the 28x28 reshapes.

Strategy (8 NeuronCores):
  - Shard the contraction dim P=50000 column-wise across 8 cores
    (6400 = 50*128 per core, zero-padded at the tail).
  - Host-side: transpose each shard to [K, 784], split fp32 into bf16
    hi + bf16 lo planes (same total bytes as fp32, so the memory
    roofline is unchanged, but the PE runs at bf16 rate instead of the
    4x-slower fp32 mode), and pack pairs of 128-row k-subtiles
    side-by-side so each DMA moves a fully contiguous [128 x 3136B]
    block (802 KB). p is split the same way; products
    hi*hi + hi*lo + lo*hi are accumulated in fp32 PSUM, recovering
    fp32-level precision (dropped lo*lo term is ~2^-18 relative).
  - Each core streams its 39 MB of matrix data HBM->SBUF (HWDGE via
    the SP sequencer) and accumulates 6 partial row-vectors ([784]
    each) in PSUM via PE matmuls with p-columns stationary.
  - Raw Bass (no Tile): this walrus build supports only ONE sync-wait
    per DMA instruction. Each DMA carries exactly one embedded wait --
    on its own round-robin lane's predecessor -- which strictly orders
    every lane's semaphore updates (race-free counts, 8 DMAs in
    flight). All other waits are standalone engine wait_ge ops.
  - Host: sum the 8 cores' partials (the "all-reduce"), reshape, gather.
"""

import numpy as np
import ml_dtypes

import concourse.bass as bass
import concourse.mybir as mybir
from concourse.bass_utils import run_bass_kernel_spmd

N_CORES = 8
P_FULL = 50000
H = W = 28
M = H * W  # 784
KT = 50  # k-subtiles (of 128) per core
K_PER = KT * 128  # 6400
NT = KT // 2  # 25 double-tiles per plane
M2 = 2 * M  # 1568 columns per double-tile
NHALF = 392  # 784 / 2, per-PSUM-bank output chunk

BF16 = ml_dtypes.bfloat16
F32 = np.float32

B = 8  # double-tile buffers per plane (4 planes x B x 3136B/partition)
N_LANES = 8


def build_nc() -> bass.Bass:
    f32 = mybir.dt.float32
    bf16 = mybir.dt.bfloat16
    nc = bass.Bass("TRN2")
    pw_d = nc.dram_tensor("pw", [128, 2 * KT], bf16, kind="ExternalInput")
    planes_d = {
        name: nc.dram_tensor(name, [NT * 128, M2], bf16, kind="ExternalInput")
        for name in ("ihi", "ilo", "jhi", "jlo")
    }
    out_d = nc.dram_tensor("out", [6, M], f32, kind="ExternalOutput")

    tiled = {
        name: t[:, :].rearrange("(n p) m -> n p m", p=128)
        for name, t in planes_d.items()
    }

    from contextlib import ExitStack

    with ExitStack() as stk:
        pw_sb = stk.enter_context(nc.sbuf_tensor("pw_sb", [128, 2 * KT], bf16))
        streams = {
            name: stk.enter_context(
                nc.sbuf_tensor(f"s_{name}", [128, B * M2], bf16)
            )
            for name in ("ihi", "ilo", "jhi", "jlo")
        }
        o_ih = stk.enter_context(nc.sbuf_tensor("o_ih", [2, M], f32))
        o_il = stk.enter_context(nc.sbuf_tensor("o_il", [1, M], f32))
        o_jh = stk.enter_context(nc.sbuf_tensor("o_jh", [2, M], f32))
        o_jl = stk.enter_context(nc.sbuf_tensor("o_jl", [1, M], f32))
        ps = {
            ("i", "h"): tuple(
                stk.enter_context(nc.psum_tensor(f"ps_ih{c}", [2, NHALF], f32))
                for c in range(2)
            ),
            ("i", "l"): tuple(
                stk.enter_context(nc.psum_tensor(f"ps_il{c}", [1, NHALF], f32))
                for c in range(2)
            ),
            ("j", "h"): tuple(
                stk.enter_context(nc.psum_tensor(f"ps_jh{c}", [2, NHALF], f32))
                for c in range(2)
            ),
            ("j", "l"): tuple(
                stk.enter_context(nc.psum_tensor(f"ps_jl{c}", [1, NHALF], f32))
                for c in range(2)
            ),
        }
        lanes = [
            stk.enter_context(nc.semaphore(f"dml{q}")) for q in range(N_LANES)
        ]
        pe_sem = stk.enter_context(nc.semaphore("pe_sem"))
        dve_sem = stk.enter_context(nc.semaphore("dve_sem"))
        block = stk.enter_context(nc.Block())

        outs = {("i", "h"): o_ih, ("i", "l"): o_il,
                ("j", "h"): o_jh, ("j", "l"): o_jl}

        def slot_cols(n):
            s = (n % B) * M2
            return slice(s, s + M2)

        # Round-robin lane bookkeeping. Every DMA goes to lane (issue mod 8),
        # increments that lane's sem by 16, and carries exactly ONE embedded
        # wait: its own lane's previous count. That strictly orders each
        # lane's sem updates (race-free counts) while keeping 8 DMAs in
        # flight, and stays within walrus's one-sync-wait-per-DMA limit.
        lane_state = {"k": 0, "counts": [0] * N_LANES}
        dma_records = {}

        def issue_dma(sync, dst, src, record_key):
            q = lane_state["k"] % N_LANES
            lane_state["k"] += 1
            prev = lane_state["counts"][q]
            ins = sync.dma_start(dst, src).then_inc(lanes[q], 16)
            if prev > 0:
                ins.wait_op(lanes[q], 16 * prev, "sem-ge")
            lane_state["counts"][q] = prev + 1
            dma_records.setdefault(record_key, []).append((q, 16 * (prev + 1)))

        @block.sync
        def _(sync):
            issue_dma(sync, pw_sb[:, :], pw_d[:, :], ("pw",))
            for n in range(NT):
                if n >= B:
                    # slot n%B was last used by double-tile n-B; wait until
                    # the PE consumed it (pe_sem counts finished double-tiles)
                    sync.wait_ge(pe_sem, n - B + 1)
                cols = slot_cols(n)
                for name in ("ihi", "ilo", "jhi", "jlo"):
                    issue_dma(
                        sync, streams[name][:, cols], tiled[name][n, :, :],
                        ("tile", n),
                    )
            # evict results once the DVE has drained all PSUMs
            sync.wait_ge(dve_sem, 1)
            issue_dma(sync, out_d[0:2, :], o_ih[:, :], ("out",))
            issue_dma(sync, out_d[2:3, :], o_il[:, :], ("out",))
            issue_dma(sync, out_d[3:5, :], o_jh[:, :], ("out",))
            issue_dma(sync, out_d[5:6, :], o_jl[:, :], ("out",))
            for q, v in dma_records[("out",)]:
                sync.wait_ge(lanes[q], v)

        @block.tensor
        def _(tensor):
            for n in range(NT):
                if n == 0:
                    for q, v in dma_records[("pw",)]:
                        tensor.wait_ge(lanes[q], v)
                for q, v in dma_records[("tile", n)]:
                    tensor.wait_ge(lanes[q], v)
                cols = slot_cols(n)
                last = None
                for two in range(2):
                    s = 2 * n + two  # k-subtile index
                    start = s == 0
                    stop = s == KT - 1
                    w2 = pw_sb[:, 2 * s : 2 * s + 2]  # [128, 2] (p_hi, p_lo)
                    w1 = pw_sb[:, 2 * s : 2 * s + 1]  # [128, 1] (p_hi)
                    for mat in ("i", "j"):
                        for c in range(2):
                            cs = slice(
                                cols.start + two * M + c * NHALF,
                                cols.start + two * M + (c + 1) * NHALF,
                            )
                            last = nc.tensor.matmul(
                                ps[(mat, "h")][c][:, :], w2,
                                streams[f"{mat}hi"][:, cs],
                                start=start, stop=stop,
                            )
                            last = nc.tensor.matmul(
                                ps[(mat, "l")][c][:, :], w1,
                                streams[f"{mat}lo"][:, cs],
                                start=start, stop=stop,
                            )
                last.then_inc(pe_sem, 1)

        @block.vector
        def _(vector):
            vector.wait_ge(pe_sem, NT)
            last = None
            for mat in ("i", "j"):
                for hl in ("h", "l"):
                    for c in range(2):
                        cs = slice(c * NHALF, (c + 1) * NHALF)
                        last = nc.vector.tensor_copy(
                            outs[(mat, hl)][:, cs], ps[(mat, hl)][c][:, :]
                        )
            last.then_inc(dve_sem, 1)

    return nc


_NC_CACHE = None


def get_nc() -> bass.Bass:
    global _NC_CACHE
    if _NC_CACHE is None:
        _NC_CACHE = build_nc()
    return _NC_CACHE


def _split_hi_lo(a32: np.ndarray):
    hi = a32.astype(BF16)
    lo = (a32 - hi.astype(F32)).astype(BF16)
    return hi, lo


def _pack_pairs(plane: np.ndarray) -> np.ndarray:
    """[K_PER, M] -> [NT*128, 2*M]: subtiles 2n,2n+1 side by side so one
    DMA moves a fully contiguous [128 x 3136B] block."""
    return np.ascontiguousarray(
        plane.reshape(NT, 2, 128, M).transpose(0, 2, 1, 3).reshape(NT * 128, M2)
    )


def shard_inputs(p, I, J) -> list[dict]:
    p = np.asarray(p, dtype=F32)
    I = np.asarray(I, dtype=F32)
    J = np.asarray(J, dtype=F32)

    p_pad = np.zeros(N_CORES * K_PER, dtype=F32)
    p_pad[:P_FULL] = p

    in_maps = []
    for c in range(N_CORES):
        lo_k = c * K_PER
        hi_k = min(lo_k + K_PER, P_FULL)
        kc = hi_k - lo_k

        pc = p_pad[c * K_PER : (c + 1) * K_PER]
        phi, plo = _split_hi_lo(pc)
        pw = np.zeros((128, 2 * KT), dtype=BF16)
        pw[:, 0::2] = phi.reshape(KT, 128).T
        pw[:, 1::2] = plo.reshape(KT, 128).T

        im = {"pw": pw}
        for name, mat in (("i", I), ("j", J)):
            t = np.zeros((K_PER, M), dtype=F32)
            if kc > 0:
                t[:kc] = mat[:, lo_k:hi_k].T
            hi_p, lo_p = _split_hi_lo(t)
            im[f"{name}hi"] = _pack_pairs(hi_p)
            im[f"{name}lo"] = _pack_pairs(lo_p)
        in_maps.append(im)
    return in_maps


def run(p, I, J, inds1, inds2, trace=False, **run_kwargs):
    """Returns ((dgm1, dgm2), BassKernelResults)."""
    in_maps = shard_inputs(p, I, J)
    nc = get_nc()
    res = run_bass_kernel_spmd(
        nc, in_maps, list(range(N_CORES)), trace=trace, **run_kwargs
    )
    acc = np.zeros((6, M), dtype=np.float64)
    for r in res.results:
        acc += r["out"].astype(np.float64)
    Xp = (acc[0] + acc[1] + acc[2]).astype(F32).reshape(H, W)
    Yp = (acc[3] + acc[4] + acc[5]).astype(F32).reshape(H, W)
    inds1 = np.asarray(inds1)
    inds2 = np.asarray(inds2)
    dgm1 = Xp[inds1[:, 0], inds1[:, 1]].reshape(-1, 2)
    dgm2 = Yp[inds2[:, 0], inds2[:, 1]].reshape(-1, 2)
    return (dgm1, dgm2), res


def kernel(p, I, J, inds1, inds2):
    out, _ = run(p, I, J, inds1, inds2, trace=False)
    return out
